# revision 2
# baseline (speedup 1.0000x reference)
"""Trainium2 Bass kernel for batched dot-product attention with query-row
masking (nn_DotProductAttention).

Problem (hardcoded): B=16, N=2048, D=128, fp32.
  scores = Q @ K^T / sqrt(D)                  [B, N, N]
  scores[b, q, :] = -1e6  where q >= valid_lens[b]   (masks whole query ROWS)
  attn = softmax(scores, axis=-1)
  out = attn @ V                              [B, N, D]

A fully-masked row softmaxes to the uniform distribution, so multiplying Q
rows by a 0/1 mask (making the score row constant 0 -> exp=1) produces the
identical result: out = mean(V) for masked rows. Two consequences used here:

1. Plain softmax without max subtraction (scores ~ N(0,1): exp never
   overflows).
2. Rows q >= valid_len need no attention compute at all. Each core gets two
   batches (slot A / slot B); per slot only `cap` 128-row query blocks are
   processed (cap chosen so all valid rows are covered, +1 padding block
   whose output rows all equal mean(V)); rows beyond cap*128 are written by
   re-DMAing the padding block's output (broadcast of mean(V)).

Per-core layout (2 batches/core across 8 cores; sorted so big-valid batches
share a slot => small static capacity for the other slot):
  S^T[k,q] = kt_c^T-stationary matmul over qt (both bf16, built via PE
  transposes), exp on ACT (PSUM->SBUF, bf16), O^T[d,q] accumulated in PSUM
  with V_c stationary. Row sums via elementwise chunk accumulation (DVE +
  gpsimd split, bf16 2x mode) + one ones-matmul per group. Normalize with
  reciprocal sums, PE-transpose back to [q,d], DMA out.

Engine budget per 1024-q group (16 key chunks): ACT 16 exps ~17.1us (the
floor), PE S+O+transposes ~16us, DVE esum+tails ~14us, Pool converts+esum
~11us. Emission is software-pipelined: S(c+1) is issued before O(c) so the
tensor engine never idles behind the ACT exp chain.
"""

import os

os.environ.setdefault("JAX_PLATFORMS", "")

import math

import numpy as np

import concourse.bass as bass
import concourse.mybir as mybir
import concourse.tile as tile
from concourse import bacc
from concourse.bass import ts
from concourse.bass_utils import run_bass_kernel_spmd
from concourse.masks import make_identity

N_CORES = 8
B = 16
N = 2048
D = 128
NB = B // N_CORES  # batches (slots) per core
KC = N // 128  # key chunks of 128
BLK = 128  # query block granularity
NBLK = N // BLK  # 16 blocks per batch
SCALE = 1.0 / math.sqrt(D)

F32 = mybir.dt.float32
BF16 = mybir.dt.bfloat16

# module-level knob: replication count used by test.py's slope timing
_REPLICATE = 1

_nc_cache = {}


def _groups(cap):
    """Split cap*128 query rows into PSUM-bank-pair groups of <=1024."""
    total = cap * BLK
    out = []
    off = 0
    while off < total:
        w = min(1024, total - off)
        out.append((off, w))
        off += w
    return out


def _col_splits(gw):
    """Split a group width into <=512-col matmul spans (PSUM bank rule)."""
    out = []
    off = 0
    while off < gw:
        w = min(512, gw - off)
        out.append((off, w))
        off += w
    return out


def build_program(replicate=1, caps=(16, 16), pool_chunks=4):
    caps = tuple(int(c) for c in caps)
    assert all(1 <= c <= 16 for c in caps)
    nc = bacc.Bacc("TRN2", target_bir_lowering=False, debug=False, num_devices=N_CORES)

    q_d = nc.dram_tensor("q", [NB, N, D], F32, kind="ExternalInput")
    k_d = nc.dram_tensor("k", [NB, N, D], F32, kind="ExternalInput")
    v_d = nc.dram_tensor("v", [NB, N, D], F32, kind="ExternalInput")
    m_d = nc.dram_tensor("mask", [NB, N], F32, kind="ExternalInput")
    o_d = nc.dram_tensor("out", [NB, N, D], F32, kind="ExternalOutput")

    with tile.TileContext(nc) as tc:
        with (
            tc.tile_pool(name="consts", bufs=1) as consts,
            tc.tile_pool(name="nat", bufs=2) as nat,  # fp32 natural staging
            tc.tile_pool(name="xb", bufs=2) as xb,  # bf16 natural (conv/masked)
            tc.tile_pool(name="tp", bufs=2) as tp,  # Q^T/K^T bf16
            tc.tile_pool(name="et", bufs=6) as etp,  # exp chunks bf16
            tc.tile_pool(name="es", bufs=2) as esp,  # esum accumulators
            tc.tile_pool(name="fin", bufs=2) as fin,  # rs/ont/ob fp32
            tc.tile_pool(name="psS", bufs=2, space="PSUM") as psS,  # 4 banks
            tc.tile_pool(name="psO", bufs=2, space="PSUM") as psO,  # 4 banks
        ):
            ident_b = consts.tile([128, 128], BF16)
            make_identity(nc, ident_b[:])
            ident_f = consts.tile([128, 128], F32)
            make_identity(nc, ident_f[:])
            ones_b = consts.tile([128, 128], BF16)
            nc.vector.memset(ones_b[:], 1.0)

            def emit_dmas(s, store):
                cap = caps[s]
                qnat = nat.tile([128, 16, D], F32, tag="qnat")
                nc.sync.dma_start(
                    qnat[:, :cap, :],
                    q_d[s, : cap * BLK, :].rearrange("(c p) d -> p c d", p=128),
                )
                knat = nat.tile([128, KC, D], F32, tag="knat")
                nc.sync.dma_start(
                    knat[:], k_d[s].rearrange("(c p) d -> p c d", p=128)
                )
                vs = nat.tile([128, KC, D], F32, tag="vs")
                nc.sync.dma_start(vs[:], v_d[s].rearrange("(c p) d -> p c d", p=128))
                mk = nat.tile([128, 16], F32, tag="mk")
                nc.sync.dma_start(
                    mk[:, :cap], m_d[s, : cap * BLK].rearrange("(c p) -> p c", p=128)
                )
                store["dma", s] = (qnat, knat, vs, mk)

            def make_prep(s, store):
                """Closure list: bf16 converts (Pool), PE transposes, DVE
                copies building kt/qt/vb for slab s. Popped a few per chunk."""
                cap = caps[s]
                qnat, knat, vs, mk = store["dma", s]
                vb = xb.tile([128, KC, D], BF16, tag="vb")
                kbuf = xb.tile([128, KC, D], BF16, tag="kbuf")
                qm = xb.tile([128, 16, D], BF16, tag="qm")
                kt = tp.tile([128, KC, 128], BF16, tag="kt")  # [d, c, k]
                qt = tp.tile([128, 16 * 128], BF16, tag="qt")  # [d, q]
                store["mm", s] = (vb, kt, qt)

                ops = []
                for c in range(KC):
                    ops.append(
                        lambda c=c: nc.gpsimd.tensor_copy(vb[:, c, :], vs[:, c, :])
                    )
                # K: convert then transpose in groups of 8 tiles
                for g in range(2):
                    for j in range(8):
                        c = 8 * g + j
                        ops.append(
                            lambda c=c: nc.gpsimd.tensor_copy(
                                kbuf[:, c, :], knat[:, c, :]
                            )
                        )

                    def kgrp(g=g):
                        pk = psS.tile([128, 1024], BF16, tag="st")
                        for j in range(8):
                            c = 8 * g + j
                            nc.tensor.transpose(
                                pk[:, ts(j, 128)], kbuf[:, c, :], ident_b[:]
                            )
                        nc.vector.tensor_copy(
                            kt[:, ts(g, 8), :].rearrange("p c k -> p (c k)"), pk[:]
                        )

                    ops.append(kgrp)
                # Q: mask-multiply (Pool) then transpose
                qgroups = [
                    list(range(g0, min(g0 + 8, cap))) for g0 in range(0, cap, 8)
                ]
                for grp in qgroups:
                    for c in grp:
                        ops.append(
                            lambda c=c: nc.gpsimd.tensor_scalar_mul(
                                qm[:, c, :], qnat[:, c, :], mk[:, c : c + 1]
                            )
                        )

                    def qgrp(grp=tuple(grp)):
                        pq = psS.tile([128, 1024], BF16, tag="st")
                        for j, c in enumerate(grp):
                            nc.tensor.transpose(
                                pq[:, ts(j, 128)], qm[:, c, :], ident_b[:]
                            )
                        w = len(grp) * 128
                        nc.vector.tensor_copy(
                            qt[:, grp[0] * 128 : grp[0] * 128 + w], pq[:, :w]
                        )

                    ops.append(qgrp)
                return ops

            def emit_slab(s, store, prep_ops):
                """Main attention loops for slab s, popping prep_ops (next
                slab's preparation) between chunks."""
                cap = caps[s]
                vb, kt, qt = store["mm", s]
                groups = _groups(cap)
                last_ob = None
                last_w = None
                per_chunk = (
                    max(1, -(-len(prep_ops) // (len(groups) * KC)))
                    if prep_ops
                    else 0
                )

                for goff, gw in groups:
                    splits = _col_splits(gw)
                    ot = psO.tile([128, 1024], F32, tag="ot")
                    esd = esp.tile([128, 1024], BF16, tag="esd")
                    esu = esp.tile([128, 1024], BF16, tag="esu")
                    ets = [None] * KC
                    sts = [None] * KC

                    def emit_S(c):
                        st = psS.tile([128, 1024], F32, tag="st")
                        sts[c] = st
                        for h0, hw in splits:
                            nc.tensor.matmul(
                                st[:, h0 : h0 + hw],
                                kt[:, c, :],
                                qt[:, goff + h0 : goff + h0 + hw],
                                start=True,
                                stop=True,
                            )

                    emit_S(0)
                    for c in range(KC):
                        et = etp.tile([128, 1024], BF16, tag="et")
                        ets[c] = et
                        nc.scalar.activation(
                            et[:, :gw],
                            sts[c][:, :gw],
                            mybir.ActivationFunctionType.Exp,
                            scale=SCALE,
                        )
                        if c + 1 < KC:
                            emit_S(c + 1)
                        first, last = c == 0, c == KC - 1
                        for h0, hw in splits:
                            nc.tensor.matmul(
                                ot[:, h0 : h0 + hw],
                                vb[:, c, :],
                                et[:, h0 : h0 + hw],
                                start=first,
                                stop=last,
                            )
                        # elementwise chunk-sum accumulation (bf16 2x mode)
                        if c < pool_chunks:
                            eng, acc, isfirst = nc.gpsimd, esu, c == 0
                        else:
                            eng, acc, isfirst = nc.vector, esd, c == pool_chunks
                        if isfirst:
                            eng.tensor_copy(acc[:, :gw], et[:, :gw])
                        else:
                            eng.tensor_add(acc[:, :gw], acc[:, :gw], et[:, :gw])
                        for _ in range(per_chunk):
                            if prep_ops:
                                prep_ops.pop(0)()

                    # ---- group tail ----
                    if pool_chunks > 0:
                        nc.vector.tensor_add(esd[:, :gw], esd[:, :gw], esu[:, :gw])
                    smb = psS.tile([128, 1024], F32, tag="st")
                    for h0, hw in splits:
                        nc.tensor.matmul(
                            smb[:, h0 : h0 + hw],
                            ones_b[:],
                            esd[:, h0 : h0 + hw],
                            start=True,
                            stop=True,
                        )
                    rs = fin.tile([128, 1024], F32, tag="rs")
                    nc.vector.reciprocal(rs[:, :gw], smb[:, :gw])
                    ont = fin.tile([128, 1024], F32, tag="ont")
                    nc.vector.tensor_mul(ont[:, :gw], ot[:, :gw], rs[:, :gw])
                    pto = psS.tile([128, 1024], F32, tag="st")
                    nblk = gw // BLK
                    for j in range(nblk):
                        nc.tensor.transpose(
                            pto[:, ts(j, 128)], ont[:, ts(j, 128)], ident_f[:]
                        )
                    ob = fin.tile([128, 8, 128], F32, tag="ob")
                    nc.vector.tensor_copy(
                        ob[:, :nblk, :], pto[:, :gw].rearrange("p (j d) -> p j d", d=128)
                    )
                    nc.sync.dma_start(
                        o_d[s, goff : goff + gw, :].rearrange(
                            "(j p) d -> p j d", p=128
                        ),
                        ob[:, :nblk, :],
                    )
                    last_ob, last_w = ob, nblk

                while prep_ops:
                    prep_ops.pop(0)()

                # rows beyond cap*128 are fully masked: replicate the padding
                # block's output rows (= mean(V)) into them
                for blk in range(cap, 16):
                    nc.sync.dma_start(
                        o_d[s, blk * BLK : (blk + 1) * BLK, :].rearrange(
                            "(j p) d -> p j d", p=128
                        ),
                        last_ob[:, last_w - 1 : last_w, :],
                    )

            def emit_body():
                store = {}
                emit_dmas(0, store)
                emit_dmas(1, store)
                prep0 = make_prep(0, store)
                # slab 0 prep runs up front (overlaps the previous For_i
                # iteration's tail at runtime via pool rotation)
                while prep0:
                    prep0.pop(0)()
                prep1 = make_prep(1, store)
                emit_slab(0, store, prep1)
                emit_slab(1, store, [])

            if replicate == 1:
                emit_body()
            else:
                with tc.For_i(0, replicate, 1):
                    emit_body()

    nc.compile()
    return nc


def plan_from_valid_lens(valid_lens):
    """Sort batches by valid length; 8 largest -> slot A, 8 smallest ->
    slot B. Returns (order, caps): order[core] = (batchA, batchB); caps =
    (capA, capB) where cap covers all valid blocks +1 padding block (the
    mean(V) broadcast source) when below 16."""
    valid_lens = np.asarray(valid_lens)
    blocks = np.ceil(valid_lens / BLK).astype(int)
    srt = np.argsort(-blocks, kind="stable")
    a_idx, b_idx = srt[:8], srt[15:7:-1]

    def slot_cap(idx):
        mx = int(blocks[idx].max())
        return mx if mx >= 16 else mx + 1

    caps = (slot_cap(a_idx), slot_cap(b_idx))
    order = [(int(a_idx[i]), int(b_idx[i])) for i in range(8)]
    return order, caps


def _make_in_maps(queries, keys, values, valid_lens):
    queries = np.asarray(queries, dtype=np.float32)
    keys = np.asarray(keys, dtype=np.float32)
    values = np.asarray(values, dtype=np.float32)
    valid_lens = np.asarray(valid_lens, dtype=np.int32)
    mask = (np.arange(N)[None, :] < valid_lens[:, None]).astype(np.float32)
    order, caps = plan_from_valid_lens(valid_lens)
    in_maps = []
    for core in range(N_CORES):
        sel = list(order[core])
        in_maps.append(
            {
                "q": np.ascontiguousarray(queries[sel]),
                "k": np.ascontiguousarray(keys[sel]),
                "v": np.ascontiguousarray(values[sel]),
                "mask": np.ascontiguousarray(mask[sel]),
            }
        )
    return in_maps, order, caps


def kernel(queries, keys, values, valid_lens):
    in_maps, order, caps = _make_in_maps(queries, keys, values, valid_lens)
    key = (_REPLICATE, caps)
    if key not in _nc_cache:
        _nc_cache[key] = build_program(_REPLICATE, caps=caps)
    nc = _nc_cache[key]
    res = run_bass_kernel_spmd(nc, in_maps, core_ids=list(range(N_CORES)))
    out = np.empty((B, N, D), dtype=np.float32)
    for core in range(N_CORES):
        for slot in range(NB):
            out[order[core][slot]] = res.results[core]["out"][slot]
    return out


# revision 18
# speedup vs baseline: 1.6170x; 1.6170x over previous
"""Trainium2 Bass kernel for batched dot-product attention with query-row
masking (nn_DotProductAttention).

Problem (hardcoded): B=16, N=2048, D=128, fp32.
  scores = Q @ K^T / sqrt(D)                  [B, N, N]
  scores[b, q, :] = -1e6  where q >= valid_lens[b]   (masks whole query ROWS)
  attn = softmax(scores, axis=-1)
  out = attn @ V                              [B, N, D]

A fully-masked row softmaxes to the uniform distribution, so multiplying Q
rows by a 0/1 mask (making the score row constant 0 -> exp=1) produces the
identical result: out = mean(V) for masked rows. Two consequences used here:

1. Plain softmax without max subtraction (scores ~ N(0,1): exp never
   overflows).
2. Rows q >= valid_len need no attention compute at all. Each core gets two
   batches (slot A / slot B); per slot only `cap` 128-row query blocks are
   processed (cap chosen so all valid rows are covered, +1 padding block
   whose output rows all equal mean(V)); rows beyond cap*128 are written by
   re-DMAing the padding block's output (broadcast of mean(V)).

Per-core layout (2 batches/core across 8 cores; sorted so big-valid batches
share a slot => small static capacity for the other slot):
  S^T[k,q] = kt_c^T-stationary matmul over qt (both bf16, built via PE
  transposes), exp on ACT (PSUM->SBUF, bf16), O^T[d,q] accumulated in PSUM
  with V_c stationary. Row sums via elementwise chunk accumulation (DVE +
  gpsimd split, bf16 2x mode) + one ones-matmul per group. Normalize with
  reciprocal sums, PE-transpose back to [q,d], DMA out.

Engine budget per 1024-q group (16 key chunks): ACT 16 exps ~17.1us (the
floor), PE S+O+transposes ~16us, DVE esum+tails ~14us, Pool converts+esum
~11us. Emission is software-pipelined: S(c+1) is issued before O(c) so the
tensor engine never idles behind the ACT exp chain.
"""

import os

os.environ.setdefault("JAX_PLATFORMS", "")

import math

import numpy as np

import concourse.bass as bass
import concourse.mybir as mybir
import concourse.tile as tile
from concourse import bacc
from concourse.bass import ts
from concourse.bass_utils import run_bass_kernel_spmd
from concourse.masks import make_identity

N_CORES = 8
B = 16
N = 2048
D = 128
NB = B // N_CORES  # batches (slots) per core
KC = N // 128  # key chunks of 128
BLK = 128  # query block granularity
NBLK = N // BLK  # 16 blocks per batch
SCALE = 1.0 / math.sqrt(D)

F32 = mybir.dt.float32
BF16 = mybir.dt.bfloat16

# module-level knob: replication count used by test.py's slope timing
_REPLICATE = 1

_nc_cache = {}


def _groups(cap):
    """Split cap*128 query rows into PSUM-bank-pair groups of <=1024."""
    total = cap * BLK
    out = []
    off = 0
    while off < total:
        w = min(1024, total - off)
        out.append((off, w))
        off += w
    return out


def _col_splits(gw):
    """Split a group width into <=512-col matmul spans (PSUM bank rule)."""
    out = []
    off = 0
    while off < gw:
        w = min(512, gw - off)
        out.append((off, w))
        off += w
    return out


def build_program(replicate=1, caps=(16, 16), pool_chunks=0, unroll=1):
    caps = tuple(int(c) for c in caps)
    assert all(1 <= c <= 16 for c in caps)
    nc = bacc.Bacc("TRN2", target_bir_lowering=False, debug=False, num_devices=N_CORES)

    q_d = nc.dram_tensor("q", [NB, N, D], F32, kind="ExternalInput")
    k_d = nc.dram_tensor("k", [NB, N, D], F32, kind="ExternalInput")
    v_d = nc.dram_tensor("v", [NB, N, D], F32, kind="ExternalInput")
    m_d = nc.dram_tensor("mask", [NB, N], F32, kind="ExternalInput")
    o_d = nc.dram_tensor("out", [NB, N, D], F32, kind="ExternalOutput")
    # bf16 DRAM scratch for the XBAR DMA transposes
    kS_d = nc.dram_tensor("kS", [NB, N, D], BF16, kind="Internal")
    qS_d = nc.dram_tensor("qS", [NB, N, D], BF16, kind="Internal")

    with tile.TileContext(nc) as tc:
        with (
            tc.tile_pool(name="consts", bufs=1) as consts,
            tc.tile_pool(name="nat", bufs=2) as nat,  # fp32 natural staging
            tc.tile_pool(name="xb", bufs=2) as xb,  # bf16 natural (conv/masked)
            tc.tile_pool(name="tp", bufs=2) as tp,  # Q^T/K^T bf16
            tc.tile_pool(name="et", bufs=6) as etp,  # exp chunks bf16
            tc.tile_pool(name="es", bufs=2) as esp,  # esum accumulators
            tc.tile_pool(name="fin", bufs=2) as fin,  # rs/ont/ob fp32
            tc.tile_pool(name="psS", bufs=2, space="PSUM") as psS,  # 4 banks
            tc.tile_pool(name="psO", bufs=2, space="PSUM") as psO,  # 4 banks
        ):
            ident_f = consts.tile([128, 128], F32)
            make_identity(nc, ident_f[:])
            ones_b = consts.tile([128, 128], BF16)
            nc.vector.memset(ones_b[:], 1.0)

            def emit_dmas(s, store):
                cap = caps[s]
                # casting DMAs (gpsimd SWDGE): fp32 HBM -> bf16
                qb = xb.tile([128, 16, D], BF16, tag="qb")
                nc.gpsimd.dma_start(
                    qb[:, :cap, :],
                    q_d[s, : cap * BLK, :].rearrange("(c p) d -> p c d", p=128),
                )
                vb = xb.tile([128, KC, D], BF16, tag="vb")
                nc.gpsimd.dma_start(
                    vb[:], v_d[s].rearrange("(c p) d -> p c d", p=128)
                )
                mk = nat.tile([128, 16], F32, tag="mk")
                nc.sync.dma_start(
                    mk[:, :cap], m_d[s, : cap * BLK].rearrange("(c p) -> p c", p=128)
                )
                # K^T: cast to bf16 DRAM scratch, then XBAR transpose-DMA
                nc.gpsimd.dma_start(kS_d[s], k_d[s])
                kt = tp.tile([128, N], BF16, tag="kt")  # [d, k]
                nc.sync.dma_start_transpose(kt[:], kS_d[s])
                store["dma", s] = (qb, vb, mk, kt)

            def make_prep(s, store):
                """Closure list: mask-muls (DVE 2x) + Q^T DMA bounce for
                slab s. Popped a few per chunk of the previous slab."""
                cap = caps[s]
                qb, vb, mk, kt = store["dma", s]
                qm = xb.tile([128, 16, D], BF16, tag="qm")
                qt = tp.tile([128, 16 * 128], BF16, tag="qt")  # [d, q]
                store["mm", s] = (vb, kt, qt)

                ops = []
                for c in range(cap):
                    ops.append(
                        lambda c=c: nc.vector.tensor_scalar_mul(
                            qm[:, c, :], qb[:, c, :], mk[:, c : c + 1]
                        )
                    )

                def qdma():
                    nc.sync.dma_start(
                        qS_d[s, : cap * BLK, :].rearrange("(c p) d -> p c d", p=128),
                        qm[:, :cap, :],
                    )
                    nc.sync.dma_start_transpose(
                        qt[:, : cap * BLK], qS_d[s, : cap * BLK, :]
                    )

                ops.append(qdma)
                return ops

            def emit_slab(s, store, prep_ops):
                """Main attention loops for slab s, popping prep_ops (next
                slab's preparation) between chunks."""
                cap = caps[s]
                vb, kt, qt = store["mm", s]
                groups = _groups(cap)
                last_ob = None
                last_w = None
                per_chunk = (
                    max(1, -(-len(prep_ops) // (len(groups) * KC)))
                    if prep_ops
                    else 0
                )

                for goff, gw in groups:
                    splits = _col_splits(gw)
                    ot = psO.tile([128, 1024], F32, tag="ot")
                    esd = esp.tile([128, 1024], BF16, tag="esd")
                    esu = (
                        esp.tile([128, 1024], BF16, tag="esu")
                        if pool_chunks > 0
                        else None
                    )
                    ets = [None] * KC
                    sts = [None] * KC

                    def emit_S(c):
                        st = psS.tile([128, 1024], F32, tag="st")
                        sts[c] = st
                        for h0, hw in splits:
                            nc.tensor.matmul(
                                st[:, h0 : h0 + hw],
                                kt[:, ts(c, 128)],
                                qt[:, goff + h0 : goff + h0 + hw],
                                start=True,
                                stop=True,
                            )

                    emit_S(0)
                    for c in range(KC):
                        et = etp.tile([128, 1024], BF16, tag="et")
                        ets[c] = et
                        nc.scalar.activation(
                            et[:, :gw],
                            sts[c][:, :gw],
                            mybir.ActivationFunctionType.Exp,
                            scale=SCALE,
                        )
                        if c + 1 < KC:
                            emit_S(c + 1)
                        first, last = c == 0, c == KC - 1
                        for h0, hw in splits:
                            nc.tensor.matmul(
                                ot[:, h0 : h0 + hw],
                                vb[:, c, :],
                                et[:, h0 : h0 + hw],
                                start=first,
                                stop=last,
                            )
                        # elementwise chunk-sum accumulation (bf16 2x mode)
                        if c < pool_chunks:
                            eng, acc, isfirst = nc.gpsimd, esu, c == 0
                        else:
                            eng, acc, isfirst = nc.vector, esd, c == pool_chunks
                        if isfirst:
                            eng.tensor_copy(acc[:, :gw], et[:, :gw])
                        else:
                            eng.tensor_add(acc[:, :gw], acc[:, :gw], et[:, :gw])
                        for _ in range(per_chunk):
                            if prep_ops:
                                prep_ops.pop(0)()

                    # ---- group tail (smb/pto live in the psO pool so the
                    # next group's S chunks never wait on the tail chain) ----
                    if pool_chunks > 0:
                        nc.vector.tensor_add(esd[:, :gw], esd[:, :gw], esu[:, :gw])
                    smb = psO.tile([128, 1024], F32, tag="ot")
                    for h0, hw in splits:
                        nc.tensor.matmul(
                            smb[:, h0 : h0 + hw],
                            ones_b[:],
                            esd[:, h0 : h0 + hw],
                            start=True,
                            stop=True,
                        )
                    rs = fin.tile([128, 1024], F32, tag="rs")
                    nc.vector.reciprocal(rs[:, :gw], smb[:, :gw])
                    ont = fin.tile([128, 1024], F32, tag="ont")
                    nc.vector.tensor_mul(ont[:, :gw], ot[:, :gw], rs[:, :gw])
                    pto = psO.tile([128, 1024], F32, tag="ot")
                    nblk = gw // BLK
                    for j in range(nblk):
                        nc.tensor.transpose(
                            pto[:, ts(j, 128)], ont[:, ts(j, 128)], ident_f[:]
                        )
                    ob = fin.tile([128, 8, 128], F32, tag="ob")
                    nc.vector.tensor_copy(
                        ob[:, :nblk, :], pto[:, :gw].rearrange("p (j d) -> p j d", d=128)
                    )
                    nc.sync.dma_start(
                        o_d[s, goff : goff + gw, :].rearrange(
                            "(j p) d -> p j d", p=128
                        ),
                        ob[:, :nblk, :],
                    )
                    last_ob, last_w = ob, nblk

                while prep_ops:
                    prep_ops.pop(0)()

                # rows beyond cap*128 are fully masked: replicate the padding
                # block's output rows (= mean(V)) into them
                for blk in range(cap, 16):
                    nc.sync.dma_start(
                        o_d[s, blk * BLK : (blk + 1) * BLK, :].rearrange(
                            "(j p) d -> p j d", p=128
                        ),
                        last_ob[:, last_w - 1 : last_w, :],
                    )

            def emit_body():
                store = {}
                emit_dmas(0, store)
                emit_dmas(1, store)
                prep0 = make_prep(0, store)
                # slab 0 prep runs up front (overlaps the previous For_i
                # iteration's tail at runtime via pool rotation)
                while prep0:
                    prep0.pop(0)()
                prep1 = make_prep(1, store)
                emit_slab(0, store, prep1)
                emit_slab(1, store, [])

            if replicate == 1:
                for _ in range(unroll):
                    emit_body()
            else:
                with tc.For_i(0, replicate, 1):
                    emit_body()

    nc.compile()
    return nc


def plan_from_valid_lens(valid_lens):
    """Sort batches by valid length; 8 largest -> slot A, 8 smallest ->
    slot B. Returns (order, caps): order[core] = (batchA, batchB); caps =
    (capA, capB) where cap covers all valid blocks +1 padding block (the
    mean(V) broadcast source) when below 16."""
    valid_lens = np.asarray(valid_lens)
    blocks = np.ceil(valid_lens / BLK).astype(int)
    srt = np.argsort(-blocks, kind="stable")
    a_idx, b_idx = srt[:8], srt[15:7:-1]

    def slot_cap(idx):
        mx = int(blocks[idx].max())
        return mx if mx >= 16 else mx + 1

    caps = (slot_cap(a_idx), slot_cap(b_idx))
    order = [(int(a_idx[i]), int(b_idx[i])) for i in range(8)]
    return order, caps


def _make_in_maps(queries, keys, values, valid_lens):
    queries = np.asarray(queries, dtype=np.float32)
    keys = np.asarray(keys, dtype=np.float32)
    values = np.asarray(values, dtype=np.float32)
    valid_lens = np.asarray(valid_lens, dtype=np.int32)
    mask = (np.arange(N)[None, :] < valid_lens[:, None]).astype(np.float32)
    order, caps = plan_from_valid_lens(valid_lens)
    in_maps = []
    for core in range(N_CORES):
        sel = list(order[core])
        in_maps.append(
            {
                "q": np.ascontiguousarray(queries[sel]),
                "k": np.ascontiguousarray(keys[sel]),
                "v": np.ascontiguousarray(values[sel]),
                "mask": np.ascontiguousarray(mask[sel]),
            }
        )
    return in_maps, order, caps


def kernel(queries, keys, values, valid_lens):
    in_maps, order, caps = _make_in_maps(queries, keys, values, valid_lens)
    key = (_REPLICATE, caps)
    if key not in _nc_cache:
        _nc_cache[key] = build_program(_REPLICATE, caps=caps)
    nc = _nc_cache[key]
    res = run_bass_kernel_spmd(nc, in_maps, core_ids=list(range(N_CORES)))
    out = np.empty((B, N, D), dtype=np.float32)
    for core in range(N_CORES):
        for slot in range(NB):
            out[order[core][slot]] = res.results[core]["out"][slot]
    return out
